# revision 2
# baseline (speedup 1.0000x reference)
"""Trainium2 Bass kernel for nn_Attention_16801912062520.

Reference computation (jax):
    S4   = S.reshape(dps, seq, H, DK)
    S_Q  = S4 @ WQ_w.T + WQ_b
    R_K  = R4 @ WK_w.T + WK_b
    R_V  = R4 @ WV_w.T + WV_b
    beta = sum(S_Q * R_K, -1)
    out  = where(S_mas, R_V * beta, 0)

Algebraic reduction (exact): beta[b,s,h] = S[b,s,:] . qv[b,h,:] + c[b,h]
with qv[b,h,:] = WQ_w.T @ R_K[b,h,:] embedded in head h's 64-wide slice of d,
and c[b,h] = WQ_b . R_K[b,h,:].  The big projection einsum never needs to be
materialized; the kernel is memory-bound (read S + write out).

This version cuts HBM traffic ~3.6x vs the fp32 full-seq kernel:
  * rows with S_mas == 0 produce exact zeros, so only unmasked rows are
    shipped/computed (host compacts via the runtime mask, device capacity is
    derived from the data, host scatters results back into a zeros array);
  * S is pre-transposed on the host (removes on-device PE transposes);
  * device I/O is fp16 (beta is accumulated in fp32 on the PE; max rel err
    ~1e-3 vs the fp32 reference, well inside the 2e-2 gate).

Sharding: batch (dps=32) split 4-per-core across 8 cores; tiny per-batch
vectors (qv, R_V, c) are precomputed on host and shipped per core.

Device loop per 512-row super-tile (input DMA'd one batch at a time):
  8 accumulating fp16 matmuls (qv^T x S^T chunks) -> beta^T [16,<=512] ->
  ACT bias add (+fp16 downcast) -> per-128-row expand matmuls
  (beta^T x Vexp block-diag) -> ACT/DVE PSUM->SBUF fp16 copies -> DMA out.
"""

import numpy as np

H, DK = 16, 64
DPS, SEQ, D = 32, 2048, 1024
NCORES = 8
NB = DPS // NCORES          # batches per core

_CACHE = {}


def _build_nc(ncp, nb=NB):
    """ncp: compacted+padded rows per batch (multiple of 128, >= 128)."""
    import concourse.bacc as bacc
    import concourse.mybir as mybir
    from concourse.tile import TileContext
    from contextlib import ExitStack

    f32 = mybir.dt.float32
    f16 = mybir.dt.float16

    nt = ncp // 128             # 128-row subtiles per batch

    nc = bacc.Bacc("TRN2", target_bir_lowering=False, debug=False)

    SC = nc.dram_tensor("SC", [nb, 8, 128, ncp], f16, kind="ExternalInput")
    qvTh = nc.dram_tensor("qvTh", [128, nb * 8 * 16], f16, kind="ExternalInput")
    vexph = nc.dram_tensor("vexph", [16, nb * D], f16, kind="ExternalInput")
    cvech = nc.dram_tensor("cvech", [16, nb], f32, kind="ExternalInput")
    outc = nc.dram_tensor("outc", [nb, ncp, D], f16, kind="ExternalOutput")

    # supers: per batch, groups of up to 4 subtiles (<=512 rows)
    sup_bounds = []
    j = 0
    while j < nt:
        nj = min(4, nt - j)
        sup_bounds.append((j, nj))
        j += nj

    with TileContext(nc) as tc, ExitStack() as ctx:
        consts = ctx.enter_context(tc.tile_pool(name="consts", bufs=1))
        sin_pool = ctx.enter_context(tc.tile_pool(name="sin", bufs=2))
        osb_pool = ctx.enter_context(tc.tile_pool(name="osb", bufs=2))
        bsb_pool = ctx.enter_context(tc.tile_pool(name="bsb", bufs=3))
        bps_pool = ctx.enter_context(tc.tile_pool(name="bps", bufs=2, space="PSUM"))
        ops_pool = ctx.enter_context(tc.tile_pool(name="ops", bufs=2, space="PSUM"))

        # Small const loads first (cheap, and they unblock the PE warm-up
        # clump below).
        qvT_sb = consts.tile([128, nb * 8 * 16], f16)
        nc.sync.dma_start(qvT_sb[:], qvTh[:, :])
        vexp_sb = consts.tile([16, nb * D], f16)
        nc.sync.dma_start(vexp_sb[:], vexph[:, :])
        cvec_sb = consts.tile([16, nb], f32)
        nc.sync.dma_start(cvec_sb[:], cvech[:, :])

        s_srcs = [SC[b].rearrange("c p i -> p c i") for b in range(nb)]
        o_dsts = [outc[b].rearrange("(t p) d -> p t d", p=128) for b in range(nb)]

        s_in0 = sin_pool.tile([128, 8, ncp], f16, tag="s_in")
        nc.sync.dma_start(s_in0[:], s_srcs[0])

        # Warm-up clump: back-to-back matmuls under the first input DMA lift
        # the PE HAM clock gate toward 2.4 GHz.  Results are discarded.
        warm_ps = bps_pool.tile([16, 512], f32, tag="bps")
        for _ in range(16):
            nc.tensor.matmul(warm_ps[:], qvT_sb[:, 0:16], qvT_sb[:, 0:512],
                             start=True, stop=True)

        # software pipeline: A(i) = beta for super i, B(i) = expand+store.
        # Emit A(0), A(1), B(0), A(2), B(1), ..., B(last) so the PE never
        # stalls on the ACT bias/downcast between beta and expand.
        work = []
        s_ins = {}
        for b in range(nb):
            for (j0, nj) in sup_bounds:
                work.append((b, j0, nj))

        def stage_a(i):
            b, j0, nj = work[i]
            if j0 == 0:
                if b == 0:
                    s_ins[b] = s_in0
                else:
                    s_ins[b] = sin_pool.tile([128, 8, ncp], f16, tag="s_in")
                    nc.sync.dma_start(s_ins[b][:], s_srcs[b])
            s_in = s_ins[b]
            n = nj * 128
            c0 = j0 * 128
            bps = bps_pool.tile([16, 512], f32, tag="bps")
            for cg in range(8):
                lhsT = qvT_sb[:, (b * 8 + cg) * 16:(b * 8 + cg + 1) * 16]
                nc.tensor.matmul(bps[:, 0:n], lhsT, s_in[:, cg, c0:c0 + n],
                                 start=(cg == 0), stop=(cg == 7))
            bsb = bsb_pool.tile([16, 512], f16, tag="bsb")
            nc.scalar.add(bsb[:, 0:n], bps[:, 0:n], cvec_sb[:, b:b + 1])
            return bsb

        def stage_b(i, bsb):
            b, j0, nj = work[i]
            o_sup = osb_pool.tile([128, 4, D], f16, tag="o_sup")
            for j in range(nj):
                ops = ops_pool.tile([128, D], f32, tag="ops")
                lhsT = bsb[:, 128 * j:128 * (j + 1)]
                for hf in range(2):
                    rhs = vexp_sb[:, b * D + 512 * hf:b * D + 512 * (hf + 1)]
                    nc.tensor.matmul(ops[:, 512 * hf:512 * (hf + 1)],
                                     lhsT, rhs, start=True, stop=True)
                # PSUM->SBUF fp16 downcast, split across ACT and DVE
                if j % 2 == 0:
                    nc.scalar.copy(o_sup[:, j, :], ops[:])
                else:
                    nc.vector.tensor_scalar_add(o_sup[:, j, :], ops[:], 0.0)
            nc.sync.dma_start(o_dsts[b][:, j0:j0 + nj, :], o_sup[:, 0:nj, :])

        pend = None
        for i in range(len(work)):
            bsb = stage_a(i)
            if pend is not None:
                stage_b(i - 1, pend)
            pend = bsb
        stage_b(len(work) - 1, pend)

    nc.compile()
    return nc


def _host_prep(S, R, S_mas, WQ_w, WQ_b, WK_w, WK_b, WV_w, WV_b):
    """Compact unmasked rows, pre-transpose S, and build the tiny per-batch
    vectors derived from R and the dk x dk weights."""
    R4 = np.asarray(R, np.float32).reshape(DPS, H, DK)
    R_K = np.einsum("bhd,ed->bhe", R4, np.asarray(WK_w, np.float32)) + np.asarray(WK_b, np.float32)
    R_V = np.einsum("bhd,ed->bhe", R4, np.asarray(WV_w, np.float32)) + np.asarray(WV_b, np.float32)
    qv = np.einsum("ed,bhe->bhd", np.asarray(WQ_w, np.float32), R_K)      # (dps, H, DK)
    c = R_K @ np.asarray(WQ_b, np.float32)                                 # (dps, H)

    mask = np.asarray(S_mas).reshape(DPS, SEQ) != 0
    idxs = [np.flatnonzero(mask[b]) for b in range(DPS)]
    ncap = max(len(ix) for ix in idxs)
    if ncap == 0:
        return None, idxs, 0
    ncp = max(128, -(-ncap // 128) * 128)

    S16 = np.asarray(S, np.float32).astype(np.float16)

    in_maps = []
    for k in range(NCORES):
        sl = slice(k * NB, (k + 1) * NB)
        qv_c, rv_c, c_c = qv[sl], R_V[sl], c[sl]

        SC = np.zeros((NB, 1024, ncp), np.float16)
        for lb in range(NB):
            b = k * NB + lb
            ix = idxs[b]
            SC[lb, :, :len(ix)] = S16[b][ix].T
        SC = SC.reshape(NB, 8, 128, ncp)

        qvT_packed = np.zeros((NB, 8, 128, 16), np.float32)
        for h in range(H):
            cg, j = divmod(h, 2)
            qvT_packed[:, cg, 64 * j:64 * (j + 1), h] = qv_c[:, h, :]
        qvTh = np.ascontiguousarray(
            qvT_packed.transpose(2, 0, 1, 3).reshape(128, NB * 8 * 16)).astype(np.float16)

        vexp = np.zeros((NB, H, D), np.float32)
        for h in range(H):
            vexp[:, h, 64 * h:64 * (h + 1)] = rv_c[:, h, :]
        vexph = np.ascontiguousarray(
            vexp.transpose(1, 0, 2).reshape(16, NB * D)).astype(np.float16)

        cvech = np.ascontiguousarray(c_c.T).astype(np.float32)             # (16, nb)

        in_maps.append({
            "SC": SC,
            "qvTh": qvTh,
            "vexph": vexph,
            "cvech": cvech,
        })
    return in_maps, idxs, ncp


def kernel(S, R, S_mas, R_mas, WQ_w, WQ_b, WK_w, WK_b, WV_w, WV_b):
    from concourse.bass_utils import run_bass_kernel_spmd

    in_maps, idxs, ncp = _host_prep(S, R, S_mas, WQ_w, WQ_b, WK_w, WK_b,
                                    WV_w, WV_b)
    out = np.zeros((DPS, SEQ, H * DK), np.float32)
    if ncp == 0:
        return out

    key = ("nc", ncp)
    if key not in _CACHE:
        _CACHE[key] = _build_nc(ncp)
    nc = _CACHE[key]

    res = run_bass_kernel_spmd(nc, in_maps, core_ids=list(range(NCORES)))
    for k in range(NCORES):
        oc = res.results[k]["outc"]
        for lb in range(NB):
            b = k * NB + lb
            ix = idxs[b]
            out[b, ix] = oc[lb, :len(ix)].astype(np.float32)
    return out


# revision 3
# speedup vs baseline: 2.6384x; 2.6384x over previous
"""Trainium2 Bass kernel for nn_Attention_16801912062520.

Reference computation (jax):
    S4   = S.reshape(dps, seq, H, DK)
    S_Q  = S4 @ WQ_w.T + WQ_b
    R_K  = R4 @ WK_w.T + WK_b
    R_V  = R4 @ WV_w.T + WV_b
    beta = sum(S_Q * R_K, -1)
    out  = where(S_mas, R_V * beta, 0)

Algebraic reduction (exact): beta[b,s,h] = S[b,s,:] . qv[b,h,:] + c[b,h]
with qv[b,h,:] = WQ_w.T @ R_K[b,h,:] embedded in head h's 64-wide slice of d,
and c[b,h] = WQ_b . R_K[b,h,:].  The big projection einsum never needs to be
materialized; the kernel is memory-bound (read S + write out).

This version cuts HBM traffic ~3.6x vs the fp32 full-seq kernel:
  * rows with S_mas == 0 produce exact zeros, so only unmasked rows are
    shipped/computed (host compacts via the runtime mask, device capacity is
    derived from the data, host scatters results back into a zeros array);
  * S is pre-transposed on the host (removes on-device PE transposes);
  * device I/O is fp16 (beta is accumulated in fp32 on the PE; max rel err
    ~1e-3 vs the fp32 reference, well inside the 2e-2 gate).

Sharding: batch (dps=32) split 4-per-core across 8 cores; tiny per-batch
vectors (qv, R_V, c) are precomputed on host and shipped per core.

Device loop per 512-row super-tile (input DMA'd one batch at a time):
  8 accumulating fp16 matmuls (qv^T x S^T chunks) -> beta^T [16,<=512] ->
  ACT bias add (+fp16 downcast) -> per-128-row expand matmuls
  (beta^T x Vexp block-diag) -> ACT/DVE PSUM->SBUF fp16 copies -> DMA out.
"""

import numpy as np

H, DK = 16, 64
DPS, SEQ, D = 32, 2048, 1024
NCORES = 8
NB = DPS // NCORES          # batches per core

_CACHE = {}


def _build_nc(ncp, nb=NB):
    """ncp: compacted+padded rows per batch (multiple of 128, >= 128)."""
    import concourse.bacc as bacc
    import concourse.mybir as mybir
    from concourse.tile import TileContext
    from contextlib import ExitStack

    f32 = mybir.dt.float32
    f16 = mybir.dt.float16

    nt = ncp // 128             # 128-row subtiles per batch

    nc = bacc.Bacc("TRN2", target_bir_lowering=False, debug=False)

    SC = nc.dram_tensor("SC", [nb, 8, 128, ncp], f16, kind="ExternalInput")
    qvTh = nc.dram_tensor("qvTh", [128, nb * 8 * 16], f16, kind="ExternalInput")
    vexph = nc.dram_tensor("vexph", [16, nb * D], f16, kind="ExternalInput")
    cvech = nc.dram_tensor("cvech", [16, nb], f32, kind="ExternalInput")
    outc = nc.dram_tensor("outc", [nb, ncp, D], f16, kind="ExternalOutput")

    # supers: per batch, groups of up to 4 subtiles (<=512 rows)
    sup_bounds = []
    j = 0
    while j < nt:
        nj = min(4, nt - j)
        sup_bounds.append((j, nj))
        j += nj

    with TileContext(nc) as tc, ExitStack() as ctx:
        consts = ctx.enter_context(tc.tile_pool(name="consts", bufs=1))
        sin_pool = ctx.enter_context(tc.tile_pool(name="sin", bufs=2))
        osb_pool = ctx.enter_context(tc.tile_pool(name="osb", bufs=2))
        bsb_pool = ctx.enter_context(tc.tile_pool(name="bsb", bufs=3))
        bps_pool = ctx.enter_context(tc.tile_pool(name="bps", bufs=2, space="PSUM"))
        ops_pool = ctx.enter_context(tc.tile_pool(name="ops", bufs=2, space="PSUM"))

        # Small const loads first (cheap, and they unblock the PE warm-up
        # clump below).
        qvT_sb = consts.tile([128, nb * 8 * 16], f16)
        nc.sync.dma_start(qvT_sb[:], qvTh[:, :])
        vexp_sb = consts.tile([16, nb * D], f16)
        nc.sync.dma_start(vexp_sb[:], vexph[:, :])
        cvec_sb = consts.tile([16, nb], f32)
        nc.sync.dma_start(cvec_sb[:], cvech[:, :])

        s_srcs = [SC[b].rearrange("c p i -> p c i") for b in range(nb)]
        o_dsts = [outc[b].rearrange("(t p) d -> p t d", p=128) for b in range(nb)]

        s_in0 = sin_pool.tile([128, 8, ncp], f16, tag="s_in")
        nc.sync.dma_start(s_in0[:], s_srcs[0])

        # Warm-up clump: back-to-back matmuls under the first input DMA lift
        # the PE HAM clock gate toward 2.4 GHz.  Results are discarded.
        warm_ps = bps_pool.tile([16, 512], f32, tag="bps")
        for _ in range(16):
            nc.tensor.matmul(warm_ps[:], qvT_sb[:, 0:16], qvT_sb[:, 0:512],
                             start=True, stop=True)

        # software pipeline: A(i) = beta for super i, B(i) = expand+store.
        # Emit A(0), A(1), B(0), A(2), B(1), ..., B(last) so the PE never
        # stalls on the ACT bias/downcast between beta and expand.
        work = []
        s_ins = {}
        for b in range(nb):
            for (j0, nj) in sup_bounds:
                work.append((b, j0, nj))

        def stage_a(i):
            b, j0, nj = work[i]
            if j0 == 0:
                if b == 0:
                    s_ins[b] = s_in0
                else:
                    s_ins[b] = sin_pool.tile([128, 8, ncp], f16, tag="s_in",
                                             name="s_in")
                    nc.sync.dma_start(s_ins[b][:], s_srcs[b])
            s_in = s_ins[b]
            n = nj * 128
            c0 = j0 * 128
            bps = bps_pool.tile([16, 512], f32, tag="bps")
            for cg in range(8):
                lhsT = qvT_sb[:, (b * 8 + cg) * 16:(b * 8 + cg + 1) * 16]
                nc.tensor.matmul(bps[:, 0:n], lhsT, s_in[:, cg, c0:c0 + n],
                                 start=(cg == 0), stop=(cg == 7))
            bsb = bsb_pool.tile([16, 512], f16, tag="bsb")
            nc.scalar.add(bsb[:, 0:n], bps[:, 0:n], cvec_sb[:, b:b + 1])
            return bsb

        def stage_b(i, bsb):
            b, j0, nj = work[i]
            o_sup = osb_pool.tile([128, 4, D], f16, tag="o_sup")
            for j in range(nj):
                ops = ops_pool.tile([128, D], f32, tag="ops")
                lhsT = bsb[:, 128 * j:128 * (j + 1)]
                for hf in range(2):
                    rhs = vexp_sb[:, b * D + 512 * hf:b * D + 512 * (hf + 1)]
                    nc.tensor.matmul(ops[:, 512 * hf:512 * (hf + 1)],
                                     lhsT, rhs, start=True, stop=True)
                # PSUM->SBUF fp16 downcast, split across ACT and DVE
                if j % 2 == 0:
                    nc.scalar.copy(o_sup[:, j, :], ops[:])
                else:
                    nc.vector.tensor_scalar_add(o_sup[:, j, :], ops[:], 0.0)
            nc.sync.dma_start(o_dsts[b][:, j0:j0 + nj, :], o_sup[:, 0:nj, :])

        pend = None
        for i in range(len(work)):
            bsb = stage_a(i)
            if pend is not None:
                stage_b(i - 1, pend)
            pend = bsb
        stage_b(len(work) - 1, pend)

    nc.compile()
    return nc


def _host_prep(S, R, S_mas, WQ_w, WQ_b, WK_w, WK_b, WV_w, WV_b):
    """Compact unmasked rows, pre-transpose S, and build the tiny per-batch
    vectors derived from R and the dk x dk weights."""
    R4 = np.asarray(R, np.float32).reshape(DPS, H, DK)
    R_K = np.einsum("bhd,ed->bhe", R4, np.asarray(WK_w, np.float32)) + np.asarray(WK_b, np.float32)
    R_V = np.einsum("bhd,ed->bhe", R4, np.asarray(WV_w, np.float32)) + np.asarray(WV_b, np.float32)
    qv = np.einsum("ed,bhe->bhd", np.asarray(WQ_w, np.float32), R_K)      # (dps, H, DK)
    c = R_K @ np.asarray(WQ_b, np.float32)                                 # (dps, H)

    mask = np.asarray(S_mas).reshape(DPS, SEQ) != 0
    idxs = [np.flatnonzero(mask[b]) for b in range(DPS)]
    ncap = max(len(ix) for ix in idxs)
    if ncap == 0:
        return None, idxs, 0
    ncp = max(128, -(-ncap // 128) * 128)

    S16 = np.asarray(S, np.float32).astype(np.float16)

    in_maps = []
    for k in range(NCORES):
        sl = slice(k * NB, (k + 1) * NB)
        qv_c, rv_c, c_c = qv[sl], R_V[sl], c[sl]

        SC = np.zeros((NB, 1024, ncp), np.float16)
        for lb in range(NB):
            b = k * NB + lb
            ix = idxs[b]
            SC[lb, :, :len(ix)] = S16[b][ix].T
        SC = SC.reshape(NB, 8, 128, ncp)

        qvT_packed = np.zeros((NB, 8, 128, 16), np.float32)
        for h in range(H):
            cg, j = divmod(h, 2)
            qvT_packed[:, cg, 64 * j:64 * (j + 1), h] = qv_c[:, h, :]
        qvTh = np.ascontiguousarray(
            qvT_packed.transpose(2, 0, 1, 3).reshape(128, NB * 8 * 16)).astype(np.float16)

        vexp = np.zeros((NB, H, D), np.float32)
        for h in range(H):
            vexp[:, h, 64 * h:64 * (h + 1)] = rv_c[:, h, :]
        vexph = np.ascontiguousarray(
            vexp.transpose(1, 0, 2).reshape(16, NB * D)).astype(np.float16)

        cvech = np.ascontiguousarray(c_c.T).astype(np.float32)             # (16, nb)

        in_maps.append({
            "SC": SC,
            "qvTh": qvTh,
            "vexph": vexph,
            "cvech": cvech,
        })
    return in_maps, idxs, ncp


def kernel(S, R, S_mas, R_mas, WQ_w, WQ_b, WK_w, WK_b, WV_w, WV_b):
    from concourse.bass_utils import run_bass_kernel_spmd

    in_maps, idxs, ncp = _host_prep(S, R, S_mas, WQ_w, WQ_b, WK_w, WK_b,
                                    WV_w, WV_b)
    out = np.zeros((DPS, SEQ, H * DK), np.float32)
    if ncp == 0:
        return out

    key = ("nc", ncp)
    if key not in _CACHE:
        _CACHE[key] = _build_nc(ncp)
    nc = _CACHE[key]

    res = run_bass_kernel_spmd(nc, in_maps, core_ids=list(range(NCORES)))
    for k in range(NCORES):
        oc = res.results[k]["outc"]
        for lb in range(NB):
            b = k * NB + lb
            ix = idxs[b]
            out[b, ix] = oc[lb, :len(ix)].astype(np.float32)
    return out


# revision 4
# speedup vs baseline: 2.8310x; 1.0730x over previous
"""Trainium2 Bass kernel for nn_Attention_16801912062520.

Reference computation (jax):
    S4   = S.reshape(dps, seq, H, DK)
    S_Q  = S4 @ WQ_w.T + WQ_b
    R_K  = R4 @ WK_w.T + WK_b
    R_V  = R4 @ WV_w.T + WV_b
    beta = sum(S_Q * R_K, -1)
    out  = where(S_mas, R_V * beta, 0)

Algebraic reduction (exact): beta[b,s,h] = S[b,s,:] . qv[b,h,:] + c[b,h]
with qv[b,h,:] = WQ_w.T @ R_K[b,h,:] embedded in head h's 64-wide slice of d,
and c[b,h] = WQ_b . R_K[b,h,:].  The big projection einsum never needs to be
materialized; the kernel is memory-bound (read S + write out).

This version cuts HBM traffic ~3.6x vs the fp32 full-seq kernel:
  * rows with S_mas == 0 produce exact zeros, so only unmasked rows are
    shipped/computed (host compacts via the runtime mask, device capacity is
    derived from the data, host scatters results back into a zeros array);
  * S is pre-transposed on the host (removes on-device PE transposes);
  * device I/O is fp16 (beta is accumulated in fp32 on the PE; max rel err
    ~1e-3 vs the fp32 reference, well inside the 2e-2 gate).

Sharding: batch (dps=32) split 4-per-core across 8 cores; tiny per-batch
vectors (qv, R_V, c) are precomputed on host and shipped per core.

Device loop per 512-row super-tile (input DMA'd one batch at a time):
  8 accumulating fp16 matmuls (qv^T x S^T chunks) -> beta^T [16,<=512] ->
  ACT bias add (+fp16 downcast) -> per-128-row expand matmuls
  (beta^T x Vexp block-diag) -> ACT/DVE PSUM->SBUF fp16 copies -> DMA out.
"""

import numpy as np

H, DK = 16, 64
DPS, SEQ, D = 32, 2048, 1024
NCORES = 8
NB = DPS // NCORES          # batches per core

_CACHE = {}


def _build_nc(ncp, nb=NB):
    """ncp: compacted+padded rows per batch (multiple of 128, >= 128)."""
    import concourse.bacc as bacc
    import concourse.mybir as mybir
    from concourse.tile import TileContext
    from contextlib import ExitStack

    f32 = mybir.dt.float32
    f16 = mybir.dt.float16

    nt = ncp // 128             # 128-row subtiles per batch

    nc = bacc.Bacc("TRN2", target_bir_lowering=False, debug=False)

    SC = nc.dram_tensor("SC", [nb, 8, 128, ncp], f16, kind="ExternalInput")
    qvTh = nc.dram_tensor("qvTh", [128, nb * 8 * 16], f16, kind="ExternalInput")
    vexph = nc.dram_tensor("vexph", [16, nb * D], f16, kind="ExternalInput")
    cvech = nc.dram_tensor("cvech", [16, nb], f32, kind="ExternalInput")
    outc = nc.dram_tensor("outc", [nb, ncp, D], f16, kind="ExternalOutput")

    # supers: per batch, groups of up to 4 subtiles (<=512 rows)
    sup_bounds = []
    j = 0
    while j < nt:
        nj = min(4, nt - j)
        sup_bounds.append((j, nj))
        j += nj

    with TileContext(nc) as tc, ExitStack() as ctx:
        consts = ctx.enter_context(tc.tile_pool(name="consts", bufs=1))
        sin_pool = ctx.enter_context(tc.tile_pool(name="sin", bufs=4))
        osb_pool = ctx.enter_context(tc.tile_pool(name="osb", bufs=2))
        bsb_pool = ctx.enter_context(tc.tile_pool(name="bsb", bufs=3))
        bps_pool = ctx.enter_context(tc.tile_pool(name="bps", bufs=2, space="PSUM"))
        ops_pool = ctx.enter_context(tc.tile_pool(name="ops", bufs=3, space="PSUM"))

        # DMA ring split (avoids HWDGE head-of-line blocking): inputs stream
        # on the SP HWDGE ring, tiny consts on the ACT HWDGE ring, outputs on
        # the Pool SWDGE ring.  Output DMAs wait on compute; on a shared ring
        # they would stall later input DMAs queued behind them.
        qvT_sb = consts.tile([128, nb * 8 * 16], f16)
        nc.scalar.dma_start(qvT_sb[:], qvTh[:, :])
        vexp_sb = consts.tile([16, nb * D], f16)
        nc.scalar.dma_start(vexp_sb[:], vexph[:, :])
        cvec_sb = consts.tile([16, nb], f32)
        nc.scalar.dma_start(cvec_sb[:], cvech[:, :])

        s_srcs = [SC[b].rearrange("c p i -> p c i") for b in range(nb)]
        o_dsts = [outc[b].rearrange("(t p) d -> p t d", p=128) for b in range(nb)]

        # software pipeline over supers: P(i) = input DMA, A(i) = beta,
        # B(i) = expand+store.  Inputs are prefetched PF supers ahead; A/B
        # are interleaved so the PE never stalls on the ACT bias/downcast
        # between beta and expand.
        work = []
        for b in range(nb):
            for (j0, nj) in sup_bounds:
                work.append((b, j0, nj))
        PF = 3
        s_sups = {}

        def prefetch(i):
            if i >= len(work):
                return
            b, j0, nj = work[i]
            n = nj * 128
            t = sin_pool.tile([128, 8, 512], f16, tag="s_sup", name="s_sup")
            nc.sync.dma_start(t[:, :, 0:n], s_srcs[b][:, :, 128 * j0:128 * j0 + n])
            s_sups[i] = t

        prefetch(0)

        # Warm-up clump: back-to-back matmuls under the first input DMA lift
        # the PE HAM clock gate toward 2.4 GHz.  Results are discarded.
        warm_ps = bps_pool.tile([16, 512], f32, tag="bps")
        for _ in range(8):
            nc.tensor.matmul(warm_ps[:], qvT_sb[:, 0:16], qvT_sb[:, 0:512],
                             start=True, stop=True)

        for i in range(1, PF):
            prefetch(i)

        def stage_a(i):
            b, j0, nj = work[i]
            n = nj * 128
            s_sup = s_sups.pop(i)
            bps = bps_pool.tile([16, 512], f32, tag="bps")
            for cg in range(8):
                lhsT = qvT_sb[:, (b * 8 + cg) * 16:(b * 8 + cg + 1) * 16]
                nc.tensor.matmul(bps[:, 0:n], lhsT, s_sup[:, cg, 0:n],
                                 start=(cg == 0), stop=(cg == 7))
            bsb = bsb_pool.tile([16, 512], f16, tag="bsb")
            nc.scalar.add(bsb[:, 0:n], bps[:, 0:n], cvec_sb[:, b:b + 1])
            return bsb

        def stage_b(i, bsb):
            b, j0, nj = work[i]
            o_sup = osb_pool.tile([128, 4, D], f16, tag="o_sup")
            for j in range(nj):
                ops = ops_pool.tile([128, D], f32, tag="ops")
                lhsT = bsb[:, 128 * j:128 * (j + 1)]
                for hf in range(2):
                    rhs = vexp_sb[:, b * D + 512 * hf:b * D + 512 * (hf + 1)]
                    nc.tensor.matmul(ops[:, 512 * hf:512 * (hf + 1)],
                                     lhsT, rhs, start=True, stop=True)
                # PSUM->SBUF fp16 downcast, split across ACT and DVE
                if j % 2 == 0:
                    nc.scalar.copy(o_sup[:, j, :], ops[:])
                else:
                    nc.vector.tensor_scalar_add(o_sup[:, j, :], ops[:], 0.0)
            nc.gpsimd.dma_start(o_dsts[b][:, j0:j0 + nj, :], o_sup[:, 0:nj, :])

        pend = None
        for i in range(len(work)):
            prefetch(i + PF)
            bsb = stage_a(i)
            if pend is not None:
                stage_b(i - 1, pend)
            pend = bsb
        stage_b(len(work) - 1, pend)

    nc.compile()
    return nc


def _host_prep(S, R, S_mas, WQ_w, WQ_b, WK_w, WK_b, WV_w, WV_b):
    """Compact unmasked rows, pre-transpose S, and build the tiny per-batch
    vectors derived from R and the dk x dk weights."""
    R4 = np.asarray(R, np.float32).reshape(DPS, H, DK)
    R_K = np.einsum("bhd,ed->bhe", R4, np.asarray(WK_w, np.float32)) + np.asarray(WK_b, np.float32)
    R_V = np.einsum("bhd,ed->bhe", R4, np.asarray(WV_w, np.float32)) + np.asarray(WV_b, np.float32)
    qv = np.einsum("ed,bhe->bhd", np.asarray(WQ_w, np.float32), R_K)      # (dps, H, DK)
    c = R_K @ np.asarray(WQ_b, np.float32)                                 # (dps, H)

    mask = np.asarray(S_mas).reshape(DPS, SEQ) != 0
    idxs = [np.flatnonzero(mask[b]) for b in range(DPS)]
    ncap = max(len(ix) for ix in idxs)
    if ncap == 0:
        return None, idxs, 0
    ncp = max(128, -(-ncap // 128) * 128)

    S16 = np.asarray(S, np.float32).astype(np.float16)

    in_maps = []
    for k in range(NCORES):
        sl = slice(k * NB, (k + 1) * NB)
        qv_c, rv_c, c_c = qv[sl], R_V[sl], c[sl]

        SC = np.zeros((NB, 1024, ncp), np.float16)
        for lb in range(NB):
            b = k * NB + lb
            ix = idxs[b]
            SC[lb, :, :len(ix)] = S16[b][ix].T
        SC = SC.reshape(NB, 8, 128, ncp)

        qvT_packed = np.zeros((NB, 8, 128, 16), np.float32)
        for h in range(H):
            cg, j = divmod(h, 2)
            qvT_packed[:, cg, 64 * j:64 * (j + 1), h] = qv_c[:, h, :]
        qvTh = np.ascontiguousarray(
            qvT_packed.transpose(2, 0, 1, 3).reshape(128, NB * 8 * 16)).astype(np.float16)

        vexp = np.zeros((NB, H, D), np.float32)
        for h in range(H):
            vexp[:, h, 64 * h:64 * (h + 1)] = rv_c[:, h, :]
        vexph = np.ascontiguousarray(
            vexp.transpose(1, 0, 2).reshape(16, NB * D)).astype(np.float16)

        cvech = np.ascontiguousarray(c_c.T).astype(np.float32)             # (16, nb)

        in_maps.append({
            "SC": SC,
            "qvTh": qvTh,
            "vexph": vexph,
            "cvech": cvech,
        })
    return in_maps, idxs, ncp


def kernel(S, R, S_mas, R_mas, WQ_w, WQ_b, WK_w, WK_b, WV_w, WV_b):
    from concourse.bass_utils import run_bass_kernel_spmd

    in_maps, idxs, ncp = _host_prep(S, R, S_mas, WQ_w, WQ_b, WK_w, WK_b,
                                    WV_w, WV_b)
    out = np.zeros((DPS, SEQ, H * DK), np.float32)
    if ncp == 0:
        return out

    key = ("nc", ncp)
    if key not in _CACHE:
        _CACHE[key] = _build_nc(ncp)
    nc = _CACHE[key]

    res = run_bass_kernel_spmd(nc, in_maps, core_ids=list(range(NCORES)))
    for k in range(NCORES):
        oc = res.results[k]["outc"]
        for lb in range(NB):
            b = k * NB + lb
            ix = idxs[b]
            out[b, ix] = oc[lb, :len(ix)].astype(np.float32)
    return out
